# revision 5
# baseline (speedup 1.0000x reference)
"""Trainium2 Bass kernel for nn_PointPredictor (Molmo point-predictor head).

Strategy
--------
Both heavy ops are row-parallel [R, 2048] @ [2048, 256] matmuls:
  * subpatch_k: vit_features rows (B*N*P = 9216 rows)
  * patch_k:    the B*N = 2304 compacted/selected rows of x (the masked
    compaction is a pure gather, so only the selected rows are computed)
Rows are sharded evenly across the 8 NeuronCores (1152 + 288 rows per
core); the small weights are replicated. The host computes the gather
indices / positions / cos-sin rotary tables (tiny mask math), transposes
the row blocks so the contraction dim lands on SBUF partitions, and
assembles the outputs. All matmul/bias/rotary math runs on device in
fp32 (float32r streaming mode = full fp32 precision at bf16-rate).

Per core the device kernel accumulates 12 row-tiles ([128, 256] fp32 in
PSUM) over 16 K-chunks. Two row-tile groups share one PSUM bank: only
the very first matmul into a bank uses start=True (bank-wide
has_written clear); every later matmul relies on per-element
has_written bits (overwrite-on-first-touch, accumulate after).
"""

import numpy as np

B, S, N, P, D, DP, DV = 4, 2048, 576, 4, 2048, 256, 2048
THETA = 10000.0
NCORES = 8
RA = (B * N * P) // NCORES      # 1152 subpatch rows per core
RB = (B * N) // NCORES          # 288 patch rows per core
KC = D // 128                   # 16 contraction chunks
HALF = DP // 2                  # 128

# (kind, index within kind, rows) for the 12 row-tiles per core.
ROW_TILES = [("A", i, 128) for i in range(RA // 128)] + [
    ("B", 0, 128), ("B", 1, 128), ("B", 2, RB - 256)]

_CACHE = {}
LAST_RESULTS = None  # BassKernelResults of the most recent run (for test.py)


def _build_nc():
    import concourse.bacc as bacc
    import concourse.mybir as mybir
    import concourse.tile as tile
    from concourse.tile_rust import add_dep_helper

    f32 = mybir.dt.float32
    f32r = mybir.dt.float32r

    nc = bacc.Bacc("TRN2", target_bir_lowering=False, debug=False)
    vitT = nc.dram_tensor("vitT", [KC, 128, RA], f32r, kind="ExternalInput").ap()
    xgT = nc.dram_tensor("xgT", [KC, 128, RB], f32r, kind="ExternalInput").ap()
    wsubT = nc.dram_tensor("wsubT", [KC, 128, DP], f32r, kind="ExternalInput").ap()
    wpT = nc.dram_tensor("wpT", [KC, 128, DP], f32r, kind="ExternalInput").ap()
    bsub = nc.dram_tensor("bsub", [128, DP], f32, kind="ExternalInput").ap()
    bp = nc.dram_tensor("bp", [128, DP], f32, kind="ExternalInput").ap()
    cosT = nc.dram_tensor("cosT", [RB, HALF], f32, kind="ExternalInput").ap()
    sinT = nc.dram_tensor("sinT", [RB, HALF], f32, kind="ExternalInput").ap()
    sub_out = nc.dram_tensor("sub_out", [RA, DP], f32, kind="ExternalOutput").ap()
    pk_out = nc.dram_tensor("pk_out", [RB, DP], f32, kind="ExternalOutput").ap()

    with tile.TileContext(nc) as tc:
        with (
            tc.tile_pool(name="const", bufs=1) as kpool,
            tc.tile_pool(name="chunks", bufs=3) as cpool,
            tc.tile_pool(name="psacc", bufs=1, space="PSUM") as ppool,
            tc.tile_pool(name="outs", bufs=3) as opool,
        ):
            bsub_sb = kpool.tile([128, DP], f32, tag="bsub")
            nc.sync.dma_start(bsub_sb[:], bsub)
            bp_sb = kpool.tile([128, DP], f32, tag="bp")
            nc.sync.dma_start(bp_sb[:], bp)

            nbt = RB // 128 + 1  # B row-tiles (3)
            cos_sb = kpool.tile([128, nbt, HALF], f32, tag="cos")
            sin_sb = kpool.tile([128, nbt, HALF], f32, tag="sin")
            nc.sync.dma_start(
                cos_sb[:, 0:2, :], cosT[0:256].rearrange("(a p) f -> p a f", p=128))
            nc.sync.dma_start(cos_sb[0:RB - 256, 2, :], cosT[256:RB])
            nc.sync.dma_start(
                sin_sb[:, 0:2, :], sinT[0:256].rearrange("(a p) f -> p a f", p=128))
            nc.sync.dma_start(sin_sb[0:RB - 256, 2, :], sinT[256:RB])

            nbanks = (len(ROW_TILES) + 1) // 2
            ps = [ppool.tile([128, 2 * DP], f32, name=f"ps{i}", tag=f"ps{i}") for i in range(nbanks)]

            for c in range(KC):
                vt = cpool.tile([128, RA], f32r, tag="vt")
                nc.sync.dma_start(vt[:], vitT[c])
                ws = cpool.tile([128, DP], f32r, tag="ws")
                nc.sync.dma_start(ws[:], wsubT[c])
                xt = cpool.tile([128, RB], f32r, tag="xt")
                nc.sync.dma_start(xt[:], xgT[c])
                wp = cpool.tile([128, DP], f32r, tag="wp")
                nc.sync.dma_start(wp[:], wpT[c])

                for ti, (kind, idx, rows) in enumerate(ROW_TILES):
                    bank, half = divmod(ti, 2)
                    acc = ps[bank][0:rows, half * DP:(half + 1) * DP]
                    if kind == "A":
                        lhs, rhs = vt[:, idx * 128: idx * 128 + rows], ws[:]
                    else:
                        lhs, rhs = xt[:, idx * 128: idx * 128 + rows], wp[:]
                    mm = nc.tensor.matmul(
                        acc,
                        lhs,
                        rhs,
                        start=(c == 0 and half == 0),
                        stop=(c == KC - 1 and half == 1),
                    )
                    # The start=True matmul clears has_written for the WHOLE
                    # bank; the other half's first matmul must come after it
                    # (per-element bits then give overwrite-on-first-touch).
                    # Tile sees no data dep between the halves, so add one.
                    if c == 0:
                        if half == 0:
                            bank_clear[bank] = mm.ins
                        else:
                            add_dep_helper(mm.ins, bank_clear[bank], sync=False,
                                           reason="psum bank-clear before second group")

            for ti, (kind, idx, rows) in enumerate(ROW_TILES):
                bank, half = divmod(ti, 2)
                acc = ps[bank][0:rows, half * DP:(half + 1) * DP]
                if kind == "A":
                    o = opool.tile([128, DP], f32, tag="oA")
                    nc.vector.tensor_add(o[0:rows, :], acc, bsub_sb[0:rows, :])
                    nc.sync.dma_start(sub_out[idx * 128: idx * 128 + rows, :], o[0:rows, :])
                else:
                    pk = opool.tile([128, DP], f32, tag="pk")
                    nc.vector.tensor_add(pk[0:rows, :], acc, bp_sb[0:rows, :])
                    cs = cos_sb[0:rows, idx, :]
                    sn = sin_sb[0:rows, idx, :]
                    o = opool.tile([128, DP], f32, tag="oB")
                    t1 = opool.tile([128, HALF], f32, tag="t1")
                    t2 = opool.tile([128, HALF], f32, tag="t2")
                    nc.vector.tensor_mul(t1[0:rows, :], pk[0:rows, 0:HALF], cs)
                    nc.vector.tensor_mul(t2[0:rows, :], pk[0:rows, HALF:DP], sn)
                    nc.vector.tensor_sub(o[0:rows, 0:HALF], t1[0:rows, :], t2[0:rows, :])
                    nc.vector.tensor_mul(t1[0:rows, :], pk[0:rows, HALF:DP], cs)
                    nc.vector.tensor_mul(t2[0:rows, :], pk[0:rows, 0:HALF], sn)
                    nc.vector.tensor_add(o[0:rows, HALF:DP], t1[0:rows, :], t2[0:rows, :])
                    nc.sync.dma_start(pk_out[idx * 128: idx * 128 + rows, :], o[0:rows, :])

    nc.compile()
    return nc


def _get_nc():
    if "nc" not in _CACHE:
        _CACHE["nc"] = _build_nc()
    return _CACHE["nc"]


def kernel(x, vit_features, token_pooling, is_image_token,
           is_indexable_image_token, image_features_mask,
           W_patch_k, b_patch_k, W_subpatch_k, b_subpatch_k,
           no_point_vector, trace=False):
    global LAST_RESULTS
    from concourse.bass_utils import run_bass_kernel_spmd

    x = np.asarray(x, dtype=np.float32)
    vit = np.asarray(vit_features, dtype=np.float32)
    is_image_token = np.asarray(is_image_token, dtype=bool)
    is_indexable = np.asarray(is_indexable_image_token, dtype=bool)
    image_features_mask = np.asarray(image_features_mask, dtype=bool)
    W_patch_k = np.asarray(W_patch_k, dtype=np.float32)
    b_patch_k = np.asarray(b_patch_k, dtype=np.float32)
    W_subpatch_k = np.asarray(W_subpatch_k, dtype=np.float32)
    b_subpatch_k = np.asarray(b_subpatch_k, dtype=np.float32)
    no_point_vector = np.asarray(no_point_vector, dtype=np.float32)

    # ---- host-side index plumbing (exact replication of the reference) ----
    src_mask = is_image_token.reshape(-1)
    dst_mask = image_features_mask.reshape(-1)
    order = np.argsort(~src_mask, kind="stable")
    dst_rank = np.cumsum(dst_mask.astype(np.int32)) - 1
    take = order[dst_rank]                               # [B*N]
    pos = np.cumsum(is_indexable.astype(np.int32), axis=-1) - 1   # [B,S]
    pos_sel = pos.reshape(-1)[take]                      # [B*N]

    inv_freq = (1.0 / (np.float32(THETA) **
                       (np.arange(0, DP, 2, dtype=np.float32) / np.float32(DP))))
    freqs = pos_sel[:, None].astype(np.float32) * inv_freq[None, :].astype(np.float32)
    cos_t = np.cos(freqs).astype(np.float32)             # [B*N, 128]
    sin_t = np.sin(freqs).astype(np.float32)

    # ---- shard + lay out device inputs ----
    scale = np.float32(1.0 / np.sqrt(np.float32(D)))
    wsubT = np.ascontiguousarray(W_subpatch_k.T).reshape(KC, 128, DP)
    wpT = np.ascontiguousarray((W_patch_k * scale).T).reshape(KC, 128, DP)
    bsub_b = np.ascontiguousarray(np.broadcast_to(b_subpatch_k, (128, DP)))
    bp_b = np.ascontiguousarray(np.broadcast_to(b_patch_k, (128, DP)))

    vit_flat = vit.reshape(B * N * P, DV)
    xg = x.reshape(B * S, D)[take]                       # [B*N, D]

    in_maps = []
    for i in range(NCORES):
        vitT_i = np.ascontiguousarray(
            vit_flat[i * RA:(i + 1) * RA].T).reshape(KC, 128, RA)
        xgT_i = np.ascontiguousarray(
            xg[i * RB:(i + 1) * RB].T).reshape(KC, 128, RB)
        in_maps.append(dict(
            vitT=vitT_i, xgT=xgT_i, wsubT=wsubT, wpT=wpT,
            bsub=bsub_b, bp=bp_b,
            cosT=np.ascontiguousarray(cos_t[i * RB:(i + 1) * RB]),
            sinT=np.ascontiguousarray(sin_t[i * RB:(i + 1) * RB]),
        ))

    nc = _get_nc()
    bkr = run_bass_kernel_spmd(nc, in_maps, list(range(NCORES)), trace=trace)
    LAST_RESULTS = bkr
    results = bkr.results

    sub = np.concatenate([results[i]["sub_out"] for i in range(NCORES)], axis=0)
    pk = np.concatenate([results[i]["pk_out"] for i in range(NCORES)], axis=0)

    # ---- assemble outputs on host ----
    pk[~dst_mask] = 0.0
    patch_k = np.concatenate(
        [pk.reshape(B, N, DP),
         np.broadcast_to(no_point_vector, (B, 1, DP))], axis=1).astype(np.float32)
    pkm = np.where(dst_mask, is_indexable.reshape(-1)[take], False).reshape(B, N)
    patch_k_mask = np.concatenate([pkm, np.ones((B, 1), bool)], axis=1)
    subpatch_k = sub.reshape(B, N, P, DP)
    image_pos_ids = np.where(dst_mask, pos_sel, 0).reshape(B, N).astype(np.int32)
    return patch_k, patch_k_mask, subpatch_k, image_pos_ids
